# revision 23
# baseline (speedup 1.0000x reference)
"""Fused causal multi-head attention on 8 Trainium2 NeuronCores.

Problem: x[4,2048,1024], W_qkv[3072,1024], W_out[1024,1024], NH=16 heads,
HD=64, causal softmax attention + output projection (fp32 reference).

Sharding: core c = 2*b + g handles batch b (of 4) and head-group g (of 2,
8 heads each).  Each core computes Q/K/V for its heads from x[b], runs
causal attention, and multiplies its half of the attention features into
W_out, producing partial y[b] contributions (full feature width).  The
host unshards by summing the partial results per batch (standard
tensor-parallel output reduce) and concatenating over batches.

Kernel notes:
 - matmul operands are bf16 (full PE rate + fast weight load); every
   accumulation is fp32 in PSUM; softmax stats (exp input, sums,
   reciprocal) are fp32.
 - scores are computed transposed: S.T[k,q] = K_blk.T-matmul so the
   softmax denominator comes free via a ones-column appended to V and no
   PE transposes of the attention matrix are needed.
 - softmax skips max-subtraction (scores are ~N(0,1) by construction;
   exp stays well inside fp32 range).  Causal masking is multiplicative
   {0,1} applied after exp - identical result to the reference's
   additive -1e9 mask.  The last k-group of each q-chunk is >= half
   above the diagonal, so only its valid q-half is computed.
 - S.T matmuls come in same-shape pairs with one wide exp over a 2-bank
   PSUM super-tile (amortizes ACT overhead, avoids PE stationary-shape
   flips).
 - the PE is kept saturated through the attention phase by interleaving
   independent full-array work between attention groups: head-pair 0
   runs with the remaining pairs' QKV projection chains injected;
   pairs 1..3 run with the previous pair's output-projection chains
   injected.  A PE duty near 100% keeps the HAM clock gate at 2.4 GHz
   (half-idle attention otherwise locks the PE at 1.2 GHz).
 - normalization: fp32 reciprocal of the sums row, partition-broadcast
   on GpSimd, multiplied on DVE.
"""

import sys

sys.path.insert(0, "/opt/trn_rl_repo")

import numpy as np

B, T, H = 4, 2048, 1024
NH, HD = 16, 64
NCORES = 8
NHL = NH // 2          # local heads per core = 8
CW = NHL * HD          # local attention feature width = 512
TCH = 512              # t-chunk (qkv, q-chunks, y)
NT = T // TCH          # 4
KB = 128               # k block rows
NKB = T // KB          # 16
VSEG = HD + 1          # V columns + ones column = 65


def _imports():
    global bass, bacc, mybir, tile, F32, BF16, ExitStack
    import concourse.bass as bass
    import concourse.bacc as bacc
    import concourse.mybir as mybir
    from concourse import tile
    from contextlib import ExitStack
    F32 = mybir.dt.float32
    BF16 = mybir.dt.bfloat16


def build_nc():
    """Build + compile the single-core SPMD Bass program."""
    _imports()
    nc = bacc.Bacc("TRN2", target_bir_lowering=False, debug=False,
                   num_devices=NCORES)

    xT = nc.dram_tensor("xT", [H, T], BF16, kind="ExternalInput").ap()
    wqkT = nc.dram_tensor("wqkT", [H, 2 * CW], BF16, kind="ExternalInput").ap()
    wvT = nc.dram_tensor("wvT", [H, CW], BF16, kind="ExternalInput").ap()
    woT = nc.dram_tensor("woT", [CW, H], BF16, kind="ExternalInput").ap()
    masks = nc.dram_tensor("masks", [128, 3 * TCH], BF16,
                           kind="ExternalInput").ap()
    yP = nc.dram_tensor("yP", [4, H, T], F32, kind="ExternalOutput").ap()

    HC = H // 128  # 8 contraction chunks over the model dim

    with tile.TileContext(nc) as tc, ExitStack() as ctx, \
            nc.allow_low_precision(reason="bf16 matmul operands, fp32 accum"):
        mm = nc.tensor.matmul
        const = ctx.enter_context(tc.tile_pool(name="const", bufs=1))
        wpool = ctx.enter_context(tc.tile_pool(name="wpool", bufs=8))
        wop = ctx.enter_context(tc.tile_pool(name="wop", bufs=4))
        qa = ctx.enter_context(tc.tile_pool(name="qa", bufs=5))
        ktp = ctx.enter_context(tc.tile_pool(name="ktp", bufs=4))
        vp = ctx.enter_context(tc.tile_pool(name="vp", bufs=1))
        xp = ctx.enter_context(tc.tile_pool(name="xp", bufs=8))
        pts = ctx.enter_context(tc.tile_pool(name="pts", bufs=4))
        ev = ctx.enter_context(tc.tile_pool(name="ev", bufs=3))
        sm = ctx.enter_context(tc.tile_pool(name="sm", bufs=2))
        psum = ctx.enter_context(tc.tile_pool(name="psum", bufs=1, space="PSUM"))

        # ---- constants ----
        vones_f = const.tile([128, NHL], F32)
        nc.any.memset(vones_f[:], 1.0)
        mask_t = []
        m0 = const.tile([128, 2 * TCH], BF16, tag="mask0", name="mask0")
        nc.sync.dma_start(m0[:], masks[:, 0:2 * TCH])
        mask_t.append(m0)
        m1 = const.tile([128, TCH], BF16, tag="mask1", name="mask1")
        nc.sync.dma_start(m1[:], masks[:, 2 * TCH:3 * TCH])
        mask_t.append(m1)

        # ---- weights ----
        wqk = []
        for hc in range(HC):
            w = wpool.tile([128, 2 * CW], BF16, tag="w", name=f"wqk{hc}")
            nc.sync.dma_start(w[:], wqkT[hc * 128:(hc + 1) * 128, :])
            wqk.append(w)
        wv = []
        for hc in range(HC):
            w = wpool.tile([128, CW], BF16, tag="wv", name=f"wv{hc}")
            nc.sync.dma_start(w[:], wvT[hc * 128:(hc + 1) * 128, :])
            wv.append(w)
        wo = []
        for cc in range(4):
            w = wop.tile([128, H], BF16, tag="wo", name=f"wo{cc}")
            nc.sync.dma_start(w[:], woT[cc * 128:(cc + 1) * 128, :])
            wo.append(w)

        # ---- persistent activations ----
        # full xT resident (bf16): 8 x [128, T]; DMAs t-chunk-major so the
        # first projection chains can start after ~1 MB instead of 4 MB.
        xt = [xp.tile([128, T], BF16, tag="xp", name=f"xt{hc}")
              for hc in range(HC)]
        for tci in range(NT):
            ts_ = slice(tci * TCH, (tci + 1) * TCH)
            for hc in range(HC):
                nc.sync.dma_start(xt[hc][:, ts_],
                                  xT[hc * 128:(hc + 1) * 128, ts_])
        QT = [qa.tile([128, T], BF16, tag="qa", name=f"QT{i}") for i in range(4)]
        KT = [ktp.tile([128, T], BF16, tag="kt", name=f"KT{i}") for i in range(4)]
        # V, bf16, [t-block, head-major 65-wide segments (64 dims + ones col)]
        V = vp.tile([128, NKB * NHL * VSEG], BF16, name="Vsb")
        Vr = V[:].rearrange("p (tb h s) -> p tb h s", h=NHL, s=VSEG)

        # ---- chain emitters ----
        def qk_chain(r, tci):
            # QK projection chain for row-block r (pair r%4; q if r<4 else k)
            def emit():
                ts_ = slice(tci * TCH, (tci + 1) * TCH)
                ps = psum.tile([128, TCH], F32, tag="ps_qk", bufs=2,
                               name=f"psqk{r}_{tci}")
                for hc in range(HC):
                    mm(ps[:], wqk[hc][:, r * 128:(r + 1) * 128],
                       xt[hc][:, ts_], start=(hc == 0), stop=(hc == HC - 1))
                dst = QT[r] if r < 4 else KT[r - 4]
                nc.vector.tensor_copy(dst[:, ts_], ps[:])
            return emit

        def v_chain(tb):
            # V projection for t-block tb -> V sbuf (ones col from vones_f)
            def emit():
                tci, tbl = tb // 4, tb % 4
                pv = psum.tile([128, CW], F32, tag="ps_qk", bufs=2,
                               name=f"psv{tb}")
                for hc in range(HC):
                    mm(pv[:], xt[hc][:, tci * TCH + tbl * 128:
                                     tci * TCH + (tbl + 1) * 128],
                       wv[hc][:], start=(hc == 0), stop=(hc == HC - 1))
                src = pv[:].rearrange("p (h d) -> p h d", d=HD)
                nc.vector.tensor_copy(Vr[:, tb, :, 0:HD], src)
                nc.vector.tensor_copy(
                    Vr[:, tb, :, HD:VSEG],
                    vones_f[:].rearrange("p (h o) -> p h o", o=1))
            return emit

        attnT = []

        ytoggle = [0]

        def y_chain(cc, f, tci, tag="ps_qk"):
            # output-projection partial for c-chunk cc -> yP[cc]
            def emit():
                ts_ = slice(tci * TCH, (tci + 1) * TCH)
                py = psum.tile([128, TCH], F32, tag=tag, bufs=2,
                               name=f"psy{cc}_{f}_{tci}")
                mm(py[:], wo[cc][:, f * 128:(f + 1) * 128],
                   attnT[cc][:, ts_], start=True, stop=True)
                e = ev.tile([128, TCH], F32, tag="ye", name=f"yev{cc}_{f}_{tci}")
                # alternate evict engine to halve the serialization
                if ytoggle[0] % 2 == 0:
                    nc.vector.tensor_copy(e[:], py[:])
                else:
                    nc.scalar.copy(e[:], py[:])
                ytoggle[0] += 1
                nc.sync.dma_start(yP[cc, f * 128:(f + 1) * 128, ts_], e[:])
            return emit

        # ======= up-front projections: all V + pair-0 Q/K =======
        for tci in range(NT):
            for tbl in range(4):
                v_chain(tci * 4 + tbl)()
            qk_chain(0, tci)()
            qk_chain(4, tci)()

        # remaining pairs' Q/K chains get injected into pair-0 attention
        pending = [qk_chain(r, tci)
                   for r in (1, 5, 2, 6, 3, 7) for tci in range(NT)]

        # ================= attention + interleaved fill ==============
        stage = None
        for h in range(NHL):
            p, off = h // 2, 64 * (h % 2)
            if h % 2 == 0:
                a = qa.tile([128, T], BF16, tag="qa", name=f"attnT{p}")
                attnT.append(a)
                if p >= 1:
                    pending.extend(y_chain(p - 1, f, tci)
                                   for f in range(8) for tci in range(NT))
            at = attnT[p]
            # sums staging: one row per q-chunk at 32-partition offsets
            stage = sm.tile([128, TCH], F32, tag="stg", name=f"stg{h}")
            nc.any.memset(stage[:], 1.0)
            gi = 0
            for qci in range(NT):
                qs = slice(qci * TCH, (qci + 1) * TCH)
                nkb = 4 * (qci + 1)
                ngrp = nkb // 2
                ob = psum.tile([128, TCH], F32, tag="ps_ob", bufs=2,
                               name=f"ob{h}_{qci}")
                for g in range(ngrp):
                    kb0, kb1 = 2 * g, 2 * g + 1
                    dg = g - (ngrp - 2)
                    # last group of each q-chunk is >= half above the causal
                    # diagonal: compute only its valid q half [256:512)
                    w_ = TCH if dg < 1 else TCH // 2
                    q0 = 0 if dg < 1 else TCH // 2
                    qsl = slice(qci * TCH + q0, (qci + 1) * TCH)
                    sb = psum.tile([128, 2 * w_], F32, tag="ps_s", bufs=2,
                                   name=f"sb{h}_{qci}_{g}")
                    mm(sb[:, 0:w_], KT[p][off:off + 64, kb0 * KB:(kb0 + 1) * KB],
                       QT[p][off:off + 64, qsl], start=True, stop=True)
                    mm(sb[:, w_:2 * w_],
                       KT[p][off:off + 64, kb1 * KB:(kb1 + 1) * KB],
                       QT[p][off:off + 64, qsl], start=True, stop=True)
                    pt = pts.tile([128, 2 * w_], BF16, tag="pts",
                                  name=f"pt{h}_{qci}_{g}")
                    nc.scalar.activation(pt[:], sb[:],
                                         mybir.ActivationFunctionType.Exp)
                    if dg >= 0:
                        nc.vector.tensor_mul(pt[:], pt[:], mask_t[dg][:])
                    mm(ob[0:VSEG, q0:TCH], Vr[:, kb0, h, :], pt[:, 0:w_],
                       start=(kb0 == 0), stop=False)
                    mm(ob[0:VSEG, q0:TCH], Vr[:, kb1, h, :], pt[:, w_:2 * w_],
                       start=False, stop=(kb1 == nkb - 1))
                    if pending and (p >= 1 or gi % 2 == 0):
                        pending.pop(0)()
                    gi += 1
                # evict unnormalized rows + stage the sums row; the
                # normalization happens batched at the pair boundary (one
                # [8, TCH] reciprocal instead of eight 1-lane ones)
                nc.vector.tensor_copy(at[off:off + 64, qs], ob[0:64, :])
                nc.vector.tensor_copy(stage[32 * qci:32 * qci + 1, :],
                                      ob[64:65, :])
            # head tail: one batched reciprocal for the 4 staged sums rows,
            # then broadcast + in-place normalize per q-chunk
            nc.vector.reciprocal(stage[:], stage[:])
            for qq in range(NT):
                rc0 = sm.tile([1, TCH], F32, tag="rc0", name=f"rc0_{h}_{qq}")
                nc.sync.dma_start(rc0[:], stage[32 * qq:32 * qq + 1, :])
                bcs = sm.tile([128, TCH], F32, tag="bcs", name=f"bcs{h}_{qq}")
                nc.gpsimd.partition_broadcast(bcs[:], rc0[:], channels=128)
                nc.vector.tensor_mul(
                    at[off:off + 64, qq * TCH:(qq + 1) * TCH],
                    at[off:off + 64, qq * TCH:(qq + 1) * TCH],
                    bcs[off:off + 64, :])

        # ===== tail: drain leftovers + last pair's y contribution =====
        pending.extend(y_chain(3, f, tci, tag="ps_s")
                       for f in range(8) for tci in range(NT))
        for t_ in pending:
            t_()

    nc.compile()
    return nc


def make_in_maps(x, W_qkv, W_out):
    """Host-side shard prep: per-core input dict (bf16 operands)."""
    import ml_dtypes
    bf16 = ml_dtypes.bfloat16
    x = np.asarray(x, np.float32)
    W_qkv = np.asarray(W_qkv, np.float32)
    W_out = np.asarray(W_out, np.float32)
    Wq, Wk, Wv = W_qkv[0:H], W_qkv[H:2 * H], W_qkv[2 * H:3 * H]
    scale = np.float32(1.0 / np.sqrt(HD))
    kk, qq = np.meshgrid(np.arange(128), np.arange(TCH), indexing="ij")
    pat = [(qq >= j * 128 + kk).astype(np.float32) for j in range(4)]
    masks = np.concatenate(
        [pat[0], pat[1], pat[2][:, TCH // 2:], pat[3][:, TCH // 2:]],
        axis=1).astype(bf16)
    in_maps = []
    for c in range(NCORES):
        b, g = c // 2, c % 2
        rows = slice(g * CW, (g + 1) * CW)
        in_maps.append({
            "xT": np.ascontiguousarray(x[b].T).astype(bf16),
            "wqkT": np.ascontiguousarray(
                np.concatenate([Wq[rows] * scale, Wk[rows]], axis=0).T
            ).astype(bf16),
            "wvT": np.ascontiguousarray(Wv[rows].T).astype(bf16),
            "woT": np.ascontiguousarray(W_out[:, rows].T).astype(bf16),
            "masks": masks,
        })
    return in_maps


def gather_output(results):
    """results: per-core dicts with 'yP' [4, H, T] partials -> [B,T,H]."""
    out = np.empty((B, T, H), np.float32)
    for b in range(B):
        acc = results[2 * b]["yP"].sum(axis=0)
        acc += results[2 * b + 1]["yP"].sum(axis=0)
        out[b] = acc.T
    return out


_CACHE = {}


def kernel(x, W_qkv, W_out):
    from concourse.bass_utils import run_bass_kernel_spmd
    if "nc" not in _CACHE:
        _CACHE["nc"] = build_nc()
    nc = _CACHE["nc"]
    in_maps = make_in_maps(x, W_qkv, W_out)
    res = run_bass_kernel_spmd(nc, in_maps, list(range(NCORES)))
    return gather_output(res.results)


# revision 24
# speedup vs baseline: 1.0105x; 1.0105x over previous
"""Fused causal multi-head attention on 8 Trainium2 NeuronCores.

Problem: x[4,2048,1024], W_qkv[3072,1024], W_out[1024,1024], NH=16 heads,
HD=64, causal softmax attention + output projection (fp32 reference).

Sharding: core c = 2*b + g handles batch b (of 4) and head-group g (of 2,
8 heads each).  Each core computes Q/K/V for its heads from x[b], runs
causal attention, and multiplies its half of the attention features into
W_out, producing partial y[b] contributions (full feature width).  The
host unshards by summing the partial results per batch (standard
tensor-parallel output reduce) and concatenating over batches.

Kernel notes:
 - matmul operands are bf16 (full PE rate + fast weight load); every
   accumulation is fp32 in PSUM; softmax stats (exp input, sums,
   reciprocal) are fp32.
 - scores are computed transposed: S.T[k,q] = K_blk.T-matmul so the
   softmax denominator comes free via a ones-column appended to V and no
   PE transposes of the attention matrix are needed.
 - softmax skips max-subtraction (scores are ~N(0,1) by construction;
   exp stays well inside fp32 range).  Causal masking is multiplicative
   {0,1} applied after exp - identical result to the reference's
   additive -1e9 mask.  The last k-group of each q-chunk is >= half
   above the diagonal, so only its valid q-half is computed.
 - S.T matmuls come in same-shape pairs with one wide exp over a 2-bank
   PSUM super-tile (amortizes ACT overhead, avoids PE stationary-shape
   flips).
 - the PE is kept saturated through the attention phase by interleaving
   independent full-array work between attention groups: head-pair 0
   runs with the remaining pairs' QKV projection chains injected;
   pairs 1..3 run with the previous pair's output-projection chains
   injected.  A PE duty near 100% keeps the HAM clock gate at 2.4 GHz
   (half-idle attention otherwise locks the PE at 1.2 GHz).
 - normalization: fp32 reciprocal of the sums row, partition-broadcast
   on GpSimd, multiplied on DVE.
"""

import sys

sys.path.insert(0, "/opt/trn_rl_repo")

import numpy as np

B, T, H = 4, 2048, 1024
NH, HD = 16, 64
NCORES = 8
NHL = NH // 2          # local heads per core = 8
CW = NHL * HD          # local attention feature width = 512
TCH = 512              # t-chunk (qkv, q-chunks, y)
NT = T // TCH          # 4
KB = 128               # k block rows
NKB = T // KB          # 16
VSEG = HD + 1          # V columns + ones column = 65


def _imports():
    global bass, bacc, mybir, tile, F32, BF16, ExitStack
    import concourse.bass as bass
    import concourse.bacc as bacc
    import concourse.mybir as mybir
    from concourse import tile
    from contextlib import ExitStack
    F32 = mybir.dt.float32
    BF16 = mybir.dt.bfloat16


def build_nc():
    """Build + compile the single-core SPMD Bass program."""
    _imports()
    nc = bacc.Bacc("TRN2", target_bir_lowering=False, debug=False,
                   num_devices=NCORES)

    xT = nc.dram_tensor("xT", [H, T], BF16, kind="ExternalInput").ap()
    wqkT = nc.dram_tensor("wqkT", [H, 2 * CW], BF16, kind="ExternalInput").ap()
    wvT = nc.dram_tensor("wvT", [H, CW], BF16, kind="ExternalInput").ap()
    woT = nc.dram_tensor("woT", [CW, H], BF16, kind="ExternalInput").ap()
    masks = nc.dram_tensor("masks", [128, 3 * TCH], BF16,
                           kind="ExternalInput").ap()
    yP = nc.dram_tensor("yP", [4, H, T], F32, kind="ExternalOutput").ap()

    HC = H // 128  # 8 contraction chunks over the model dim

    with tile.TileContext(nc) as tc, ExitStack() as ctx, \
            nc.allow_low_precision(reason="bf16 matmul operands, fp32 accum"):
        mm = nc.tensor.matmul
        const = ctx.enter_context(tc.tile_pool(name="const", bufs=1))
        wpool = ctx.enter_context(tc.tile_pool(name="wpool", bufs=8))
        wop = ctx.enter_context(tc.tile_pool(name="wop", bufs=4))
        qa = ctx.enter_context(tc.tile_pool(name="qa", bufs=5))
        ktp = ctx.enter_context(tc.tile_pool(name="ktp", bufs=4))
        vp = ctx.enter_context(tc.tile_pool(name="vp", bufs=1))
        xp = ctx.enter_context(tc.tile_pool(name="xp", bufs=8))
        pts = ctx.enter_context(tc.tile_pool(name="pts", bufs=4))
        ev = ctx.enter_context(tc.tile_pool(name="ev", bufs=3))
        sm = ctx.enter_context(tc.tile_pool(name="sm", bufs=2))
        psum = ctx.enter_context(tc.tile_pool(name="psum", bufs=1, space="PSUM"))

        # ---- constants ----
        vones_f = const.tile([128, NHL], F32)
        nc.any.memset(vones_f[:], 1.0)
        mask_t = []
        m0 = const.tile([128, 2 * TCH], BF16, tag="mask0", name="mask0")
        nc.sync.dma_start(m0[:], masks[:, 0:2 * TCH])
        mask_t.append(m0)
        m1 = const.tile([128, TCH], BF16, tag="mask1", name="mask1")
        nc.sync.dma_start(m1[:], masks[:, 2 * TCH:3 * TCH])
        mask_t.append(m1)

        # ---- weights ----
        wqk = []
        for hc in range(HC):
            w = wpool.tile([128, 2 * CW], BF16, tag="w", name=f"wqk{hc}")
            nc.sync.dma_start(w[:], wqkT[hc * 128:(hc + 1) * 128, :])
            wqk.append(w)
        wv = []
        for hc in range(HC):
            w = wpool.tile([128, CW], BF16, tag="wv", name=f"wv{hc}")
            nc.sync.dma_start(w[:], wvT[hc * 128:(hc + 1) * 128, :])
            wv.append(w)
        wo = []
        for cc in range(4):
            w = wop.tile([128, H], BF16, tag="wo", name=f"wo{cc}")
            nc.sync.dma_start(w[:], woT[cc * 128:(cc + 1) * 128, :])
            wo.append(w)

        # ---- persistent activations ----
        # full xT resident (bf16): 8 x [128, T]; DMAs t-chunk-major so the
        # first projection chains can start after ~1 MB instead of 4 MB.
        xt = [xp.tile([128, T], BF16, tag="xp", name=f"xt{hc}")
              for hc in range(HC)]
        for tci in range(NT):
            ts_ = slice(tci * TCH, (tci + 1) * TCH)
            for hc in range(HC):
                nc.sync.dma_start(xt[hc][:, ts_],
                                  xT[hc * 128:(hc + 1) * 128, ts_])
        QT = [qa.tile([128, T], BF16, tag="qa", name=f"QT{i}") for i in range(4)]
        KT = [ktp.tile([128, T], BF16, tag="kt", name=f"KT{i}") for i in range(4)]
        # V, bf16, [t-block, head-major 65-wide segments (64 dims + ones col)]
        V = vp.tile([128, NKB * NHL * VSEG], BF16, name="Vsb")
        Vr = V[:].rearrange("p (tb h s) -> p tb h s", h=NHL, s=VSEG)

        # ---- chain emitters ----
        def qk_chain(r, tci):
            # QK projection chain for row-block r (pair r%4; q if r<4 else k)
            def emit():
                ts_ = slice(tci * TCH, (tci + 1) * TCH)
                ps = psum.tile([128, TCH], F32, tag="ps_qk", bufs=2,
                               name=f"psqk{r}_{tci}")
                for hc in range(HC):
                    mm(ps[:], wqk[hc][:, r * 128:(r + 1) * 128],
                       xt[hc][:, ts_], start=(hc == 0), stop=(hc == HC - 1))
                dst = QT[r] if r < 4 else KT[r - 4]
                nc.vector.tensor_copy(dst[:, ts_], ps[:])
            return emit

        def v_chain(tb):
            # V projection for t-block tb -> V sbuf (ones col from vones_f)
            def emit():
                tci, tbl = tb // 4, tb % 4
                pv = psum.tile([128, CW], F32, tag="ps_qk", bufs=2,
                               name=f"psv{tb}")
                for hc in range(HC):
                    mm(pv[:], xt[hc][:, tci * TCH + tbl * 128:
                                     tci * TCH + (tbl + 1) * 128],
                       wv[hc][:], start=(hc == 0), stop=(hc == HC - 1))
                src = pv[:].rearrange("p (h d) -> p h d", d=HD)
                nc.vector.tensor_copy(Vr[:, tb, :, 0:HD], src)
                nc.vector.tensor_copy(
                    Vr[:, tb, :, HD:VSEG],
                    vones_f[:].rearrange("p (h o) -> p h o", o=1))
            return emit

        attnT = []

        ytoggle = [0]

        def y_chain(cc, f, tci, tag="ps_qk"):
            # output-projection partial for c-chunk cc -> yP[cc]
            def emit():
                ts_ = slice(tci * TCH, (tci + 1) * TCH)
                py = psum.tile([128, TCH], F32, tag=tag, bufs=2,
                               name=f"psy{cc}_{f}_{tci}")
                mm(py[:], wo[cc][:, f * 128:(f + 1) * 128],
                   attnT[cc][:, ts_], start=True, stop=True)
                e = ev.tile([128, TCH], F32, tag="ye", name=f"yev{cc}_{f}_{tci}")
                # alternate evict engine to halve the serialization
                if ytoggle[0] % 2 == 0:
                    nc.vector.tensor_copy(e[:], py[:])
                else:
                    nc.scalar.copy(e[:], py[:])
                ytoggle[0] += 1
                nc.sync.dma_start(yP[cc, f * 128:(f + 1) * 128, ts_], e[:])
            return emit

        # ======= up-front projections: all V + pair-0 Q/K =======
        for tci in range(NT):
            for tbl in range(4):
                v_chain(tci * 4 + tbl)()
            qk_chain(0, tci)()
            qk_chain(4, tci)()

        # remaining pairs' Q/K chains get injected into pair-0/1 attention
        pending = [qk_chain(r, tci)
                   for r in (1, 5, 2, 6) for tci in range(NT)]

        # ================= attention + interleaved fill ==============
        stage = None
        for h in range(NHL):
            p, off = h // 2, 64 * (h % 2)
            if h % 2 == 0:
                a = qa.tile([128, T], BF16, tag="qa", name=f"attnT{p}")
                attnT.append(a)
                if p == 1:
                    pending.extend(qk_chain(r, tci)
                                   for r in (3, 7) for tci in range(NT))
                if p >= 1:
                    pending.extend(y_chain(p - 1, f, tci)
                                   for f in range(8) for tci in range(NT))
            at = attnT[p]
            # sums staging: one row per q-chunk at 32-partition offsets
            stage = sm.tile([128, TCH], F32, tag="stg", name=f"stg{h}")
            nc.any.memset(stage[:], 1.0)
            gi = 0
            for qci in range(NT):
                qs = slice(qci * TCH, (qci + 1) * TCH)
                nkb = 4 * (qci + 1)
                ngrp = nkb // 2
                ob = psum.tile([128, TCH], F32, tag="ps_ob", bufs=2,
                               name=f"ob{h}_{qci}")
                for g in range(ngrp):
                    kb0, kb1 = 2 * g, 2 * g + 1
                    dg = g - (ngrp - 2)
                    # last group of each q-chunk is >= half above the causal
                    # diagonal: compute only its valid q half [256:512)
                    w_ = TCH if dg < 1 else TCH // 2
                    q0 = 0 if dg < 1 else TCH // 2
                    qsl = slice(qci * TCH + q0, (qci + 1) * TCH)
                    sb = psum.tile([128, 2 * w_], F32, tag="ps_s", bufs=2,
                                   name=f"sb{h}_{qci}_{g}")
                    mm(sb[:, 0:w_], KT[p][off:off + 64, kb0 * KB:(kb0 + 1) * KB],
                       QT[p][off:off + 64, qsl], start=True, stop=True)
                    mm(sb[:, w_:2 * w_],
                       KT[p][off:off + 64, kb1 * KB:(kb1 + 1) * KB],
                       QT[p][off:off + 64, qsl], start=True, stop=True)
                    pt = pts.tile([128, 2 * w_], BF16, tag="pts",
                                  name=f"pt{h}_{qci}_{g}")
                    nc.scalar.activation(pt[:], sb[:],
                                         mybir.ActivationFunctionType.Exp)
                    if dg >= 0:
                        nc.vector.tensor_mul(pt[:], pt[:], mask_t[dg][:])
                    mm(ob[0:VSEG, q0:TCH], Vr[:, kb0, h, :], pt[:, 0:w_],
                       start=(kb0 == 0), stop=False)
                    mm(ob[0:VSEG, q0:TCH], Vr[:, kb1, h, :], pt[:, w_:2 * w_],
                       start=False, stop=(kb1 == nkb - 1))
                    if pending:
                        pending.pop(0)()
                    gi += 1
                # evict unnormalized rows + stage the sums row; the
                # normalization happens batched at the pair boundary (one
                # [8, TCH] reciprocal instead of eight 1-lane ones)
                nc.vector.tensor_copy(at[off:off + 64, qs], ob[0:64, :])
                nc.vector.tensor_copy(stage[32 * qci:32 * qci + 1, :],
                                      ob[64:65, :])
            # head tail: one batched reciprocal for the 4 staged sums rows,
            # then broadcast + in-place normalize per q-chunk
            nc.vector.reciprocal(stage[:], stage[:])
            for qq in range(NT):
                rc0 = sm.tile([1, TCH], F32, tag="rc0", name=f"rc0_{h}_{qq}")
                nc.sync.dma_start(rc0[:], stage[32 * qq:32 * qq + 1, :])
                bcs = sm.tile([128, TCH], F32, tag="bcs", name=f"bcs{h}_{qq}")
                nc.gpsimd.partition_broadcast(bcs[:], rc0[:], channels=128)
                nc.vector.tensor_mul(
                    at[off:off + 64, qq * TCH:(qq + 1) * TCH],
                    at[off:off + 64, qq * TCH:(qq + 1) * TCH],
                    bcs[off:off + 64, :])

        # ===== tail: drain leftovers + last pair's y contribution =====
        pending.extend(y_chain(3, f, tci, tag="ps_s")
                       for f in range(8) for tci in range(NT))
        for t_ in pending:
            t_()

    nc.compile()
    return nc


def make_in_maps(x, W_qkv, W_out):
    """Host-side shard prep: per-core input dict (bf16 operands)."""
    import ml_dtypes
    bf16 = ml_dtypes.bfloat16
    x = np.asarray(x, np.float32)
    W_qkv = np.asarray(W_qkv, np.float32)
    W_out = np.asarray(W_out, np.float32)
    Wq, Wk, Wv = W_qkv[0:H], W_qkv[H:2 * H], W_qkv[2 * H:3 * H]
    scale = np.float32(1.0 / np.sqrt(HD))
    kk, qq = np.meshgrid(np.arange(128), np.arange(TCH), indexing="ij")
    pat = [(qq >= j * 128 + kk).astype(np.float32) for j in range(4)]
    masks = np.concatenate(
        [pat[0], pat[1], pat[2][:, TCH // 2:], pat[3][:, TCH // 2:]],
        axis=1).astype(bf16)
    in_maps = []
    for c in range(NCORES):
        b, g = c // 2, c % 2
        rows = slice(g * CW, (g + 1) * CW)
        in_maps.append({
            "xT": np.ascontiguousarray(x[b].T).astype(bf16),
            "wqkT": np.ascontiguousarray(
                np.concatenate([Wq[rows] * scale, Wk[rows]], axis=0).T
            ).astype(bf16),
            "wvT": np.ascontiguousarray(Wv[rows].T).astype(bf16),
            "woT": np.ascontiguousarray(W_out[:, rows].T).astype(bf16),
            "masks": masks,
        })
    return in_maps


def gather_output(results):
    """results: per-core dicts with 'yP' [4, H, T] partials -> [B,T,H]."""
    out = np.empty((B, T, H), np.float32)
    for b in range(B):
        acc = results[2 * b]["yP"].sum(axis=0)
        acc += results[2 * b + 1]["yP"].sum(axis=0)
        out[b] = acc.T
    return out


_CACHE = {}


def kernel(x, W_qkv, W_out):
    from concourse.bass_utils import run_bass_kernel_spmd
    if "nc" not in _CACHE:
        _CACHE["nc"] = build_nc()
    nc = _CACHE["nc"]
    in_maps = make_in_maps(x, W_qkv, W_out)
    res = run_bass_kernel_spmd(nc, in_maps, list(range(NCORES)))
    return gather_output(res.results)
